# revision 24
# baseline (speedup 1.0000x reference)
"""Single-head causal attention (B=4, S=4096, D=128, fp32) on 8 Trainium2 cores.

Sharding: core c = (batch b = c//2, fold f = c%2). Each core processes ALL
queries of its batch but only the keys in 128-row chunks at global offsets
256*k + 128*f (k = 0..15). This interleaving makes the causal schedule
identical on every core (SPMD requires one program), while host-side input
prep (pre-transposed x, compacted kv rows, mask tiles passed as data) absorbs
all per-core differences into the input data.

Device outputs per core: unnormalized out^T partial [128, 4096] (bf16) and
softmax denominator partial [1, 4096] (fp32). Host combines the two folds per
batch: out[b] = ((outT0 + outT1) / (den0 + den1)).T

Dtypes: x/W/Q/K are fp16 (best mantissa for ~N(0,1) data, halves the DMA
head); exp weights / V / masks / out partials are bf16 (fp32 exponent range —
the unnormalized partials reach ~1e5 which overflows fp16). All matmuls run
at the same 1 col/cycle PE rate; 16-bit SBUF tiles unlock DVE 2x modes.

Diagonal units exploit causality: the second key chunk of a diagonal unit is
fully masked for queries 0..255 of the block on both folds, so S^T/exp/PV/den
all restrict it to columns 256:512. Its mask equals maskA[:, 0:256], so only
one [128, 512] mask tile is uploaded.

Denominator matmuls (1-column all-ones stationary) are batched per query
block: a partial-width LDWEIGHTS cannot pull ahead under an in-flight
full-width matmul (~90ns exposed stall), so paying that once per block
instead of once per unit saves ~2.5us. A full-width all-ones stationary
would avoid the stall but lights up the whole MAC array and trips the P0
power-state downclock (PE 2.4 -> 2.0 GHz; measured).
"""

import numpy as np
from contextlib import ExitStack
from ml_dtypes import bfloat16

import concourse.bacc as bacc
import concourse.tile as tile
import concourse.mybir as mybir
from concourse.bass_utils import run_bass_kernel_spmd

B, S, D = 4, 4096, 128
NCORES = 8
QB = 512          # query block (matmul moving dim)
CK = 128          # key chunk (matmul stationary dim)
NQB = S // QB     # 8 query blocks
NCK = 16          # key chunks per core (S/2/CK)
SCALE = float(1.0 / np.sqrt(D))
WARMUP_MMS = 3    # dummy matmuls to ramp the PE clock during the DMA head
DVE_DEN = {6, 7}  # blocks whose denominator accumulates on the DVE instead
                  # of streaming every exp tile through the PE a second time

FP32 = mybir.dt.float32
FP16 = mybir.dt.float16
BF16 = mybir.dt.bfloat16

_CACHE = {}


def _build():
    nc = bacc.Bacc("TRN2", target_bir_lowering=False, debug=False)

    xqT = nc.dram_tensor("xqT", [D, S], FP16, kind="ExternalInput").ap()
    xkvT = nc.dram_tensor("xkvT", [D, S // 2], FP16, kind="ExternalInput").ap()
    wpack = nc.dram_tensor("wpack", [D, 3 * D], FP16, kind="ExternalInput").ap()
    mpack = nc.dram_tensor("mpack", [CK, QB], BF16, kind="ExternalInput").ap()

    outT = nc.dram_tensor("outT", [D, S], BF16, kind="ExternalOutput").ap()
    den = nc.dram_tensor("den", [1, S], FP32, kind="ExternalOutput").ap()

    with tile.TileContext(nc) as tc, ExitStack() as ctx:
        consts = ctx.enter_context(tc.tile_pool(name="consts", bufs=1))
        stage = ctx.enter_context(tc.tile_pool(name="stage", bufs=2))
        ptp = ctx.enter_context(tc.tile_pool(name="ptp", bufs=12))
        accp = ctx.enter_context(tc.tile_pool(name="accp", bufs=2))
        ps_s = ctx.enter_context(tc.tile_pool(name="ps_s", bufs=3, space="PSUM"))
        ps_o = ctx.enter_context(tc.tile_pool(name="ps_o", bufs=1, space="PSUM"))
        ps_d = ctx.enter_context(tc.tile_pool(name="ps_d", bufs=1, space="PSUM"))

        # ---- PE warm-up: dummy matmuls on zeroed scratch, no load deps ----
        t_z = consts.tile([D, QB], FP16, tag="z")
        nc.vector.memset(t_z[:], 0.0)
        t_ones = consts.tile([CK, 1], BF16, tag="ones")
        nc.gpsimd.memset(t_ones[:], 1.0)
        # dummy activation: hoists the one-time ~2.7us exp table load off
        # the first real exp (which sits on the critical PE->ACT->PE path)
        t_warm = consts.tile([CK, 8], BF16, tag="actwarm")
        nc.scalar.activation(t_warm[:], t_z[:, 0:8],
                             mybir.ActivationFunctionType.Exp)

        def dummy_mm():
            pz = ps_s.tile([CK, 2 * QB], FP32, tag="s", name="pz")
            nc.tensor.matmul(pz[:, 0:QB], t_z[:, 0:CK], t_z[:],
                             start=True, stop=True)

        for _ in range(WARMUP_MMS):
            dummy_mm()

        # ---- loads: one queue, ordered by when compute consumes the data ----
        t_wp = consts.tile([D, 3 * D], FP16, tag="wp")
        t_xkv = consts.tile([D, S // 2], FP16, tag="xkv")
        t_xq = consts.tile([D, S], FP16, tag="xq")
        t_mA = consts.tile([CK, QB], BF16, tag="mp")

        # two parallel DMA queues; first-needed chunks are small so the
        # first projections can start ~3us earlier
        def ld(dst, src_ap):
            nc.sync.dma_start(dst, src_ap)

        def ld2(dst, src_ap):
            nc.gpsimd.dma_start(dst, src_ap)

        ld(t_wp[:, 0:2 * D], wpack[:, 0:2 * D])          # wq | wk
        ld(t_xkv[:, 0:256], xkvT[:, 0:256])
        ld(t_xkv[:, 256:512], xkvT[:, 256:512])
        ld(t_wp[:, 2 * D:3 * D], wpack[:, 2 * D:3 * D])  # wv
        ld(t_mA[:], mpack[:])
        ld(t_xkv[:, 512:1024], xkvT[:, 512:1024])
        ld(t_xkv[:, 1024:2048], xkvT[:, 1024:2048])
        ld2(t_xq[:, 512:1024], xqT[:, 512:1024])
        ld2(t_xq[:, 1024:1536], xqT[:, 1024:1536])
        ld2(t_xq[:, 1536:2048], xqT[:, 1536:2048])
        ld2(t_xq[:, 0:512], xqT[:, 0:512])
        ld2(t_xq[:, 2048:3072], xqT[:, 2048:3072])
        ld2(t_xq[:, 3072:4096], xqT[:, 3072:4096])

        t_wq = t_wp[:, 0:D]
        t_wk = t_wp[:, D:2 * D]
        t_wv = t_wp[:, 2 * D:3 * D]

        # ---- projections (emitted interleaved with attention blocks, in the
        # order the DMA queue delivers their inputs) ----
        t_KT = consts.tile([D, S // 2], FP16, tag="KT")
        t_QT = consts.tile([D, S], FP16, tag="QT")
        t_V = consts.tile([CK, NCK * D], BF16, tag="V")

        def proj_kt(t, halves=False):   # 512 compacted keys = chunks 4t..4t+3
            pk = ps_s.tile([D, 2 * QB], FP32, tag="s")
            if halves:
                for h in range(2):
                    sl = slice(t * QB + h * 256, t * QB + (h + 1) * 256)
                    nc.tensor.matmul(pk[:, h * 256:(h + 1) * 256], t_wk,
                                     t_xkv[:, sl], start=True, stop=True)
            else:
                nc.tensor.matmul(pk[:, 0:QB], t_wk,
                                 t_xkv[:, t * QB:(t + 1) * QB],
                                 start=True, stop=True)
            nc.vector.tensor_copy(t_KT[:, t * QB:(t + 1) * QB], pk[:, 0:QB])

        def proj_qt(t):   # query block t
            pq = ps_s.tile([D, 2 * QB], FP32, tag="s")
            nc.tensor.matmul(pq[:, 0:QB], t_wq, t_xq[:, t * QB:(t + 1) * QB],
                             start=True, stop=True)
            nc.vector.tensor_copy(t_QT[:, t * QB:(t + 1) * QB], pq[:, 0:QB])

        def proj_v(t):    # chunks 4t..4t+3
            pv = ps_s.tile([D, 2 * QB], FP32, tag="s")
            for h in range(4):
                k = 4 * t + h
                nc.tensor.matmul(pv[:, h * D:(h + 1) * D],
                                 t_xkv[:, k * CK:(k + 1) * CK], t_wv,
                                 start=True, stop=True)
            nc.vector.tensor_copy(t_V[:, t * QB:(t + 1) * QB], pv[:, 0:QB])

        # ---- attention: flat unit stream, software-pipelined so each
        # unit's PV matmuls are emitted after the NEXT unit's S^T matmuls
        # (PE is in-order; this hides the exp latency). Denominator matmuls
        # are batched per block (see module docstring). ----
        JORDER = [1, 2, 3, 0, 4, 5, 6, 7]
        # last block runs its diagonal group first so the final unit has no
        # mask-mul chain in the tail
        def groups(j):
            if j == JORDER[-1]:
                return [j] + list(range(j))
            return list(range(j + 1))
        units = [(j, g) for j in JORDER for g in groups(j)]
        pt_of = {}
        po_of = {}
        acc_of = {}

        # projections interleaved at block starts, matching DMA arrival
        projs_at = {
            1: [lambda: proj_kt(0, halves=True), lambda: proj_qt(1),
                lambda: proj_v(0)],
            2: [lambda: proj_kt(1), lambda: proj_qt(2), lambda: proj_v(1)],
            3: [lambda: proj_kt(2), lambda: proj_kt(3), lambda: proj_qt(3),
                lambda: proj_v(2), lambda: proj_v(3)],
            0: [lambda: proj_qt(0)],
            4: [lambda: proj_qt(4)],
            5: [lambda: proj_qt(5)],
            6: [lambda: proj_qt(6)],
            7: [lambda: proj_qt(7)],
        }

        # Diagonal unit geometry: chunk B (key offset 256+128f within the
        # block) is fully masked for block-local queries 0..255 on both
        # folds, so its scores/exp/PV/den cover only query columns 256:512.
        # Its surviving mask equals maskA[:, 0:256].
        def emit_S(u):
            j, g = units[u]
            qs = slice(j * QB, (j + 1) * QB)
            ka, kb = 2 * g, 2 * g + 1
            diag = (g == j)
            pst = ps_s.tile([CK, 2 * QB], FP32, tag="s")
            nc.tensor.matmul(pst[:, 0:QB],
                             t_KT[:, ka * CK:(ka + 1) * CK], t_QT[:, qs],
                             start=True, stop=True)
            if diag:
                qsB = slice(j * QB + 256, (j + 1) * QB)
                nc.tensor.matmul(pst[:, QB:QB + 256],
                                 t_KT[:, kb * CK:(kb + 1) * CK], t_QT[:, qsB],
                                 start=True, stop=True)
                wid = QB + 256
            else:
                nc.tensor.matmul(pst[:, QB:2 * QB],
                                 t_KT[:, kb * CK:(kb + 1) * CK], t_QT[:, qs],
                                 start=True, stop=True)
                wid = 2 * QB
            pt = ptp.tile([CK, 2 * QB], BF16, tag="pt")
            nc.scalar.activation(pt[:, 0:wid], pst[:, 0:wid],
                                 mybir.ActivationFunctionType.Exp,
                                 scale=SCALE)
            if diag:
                nc.vector.tensor_mul(pt[:, 0:QB], pt[:, 0:QB], t_mA)
                nc.vector.tensor_mul(pt[:, QB:QB + 256], pt[:, QB:QB + 256],
                                     t_mA[:, 0:256])
            pt_of[u] = pt
            # DVE-denominator blocks: fold this unit's exp tile into the
            # block accumulator as soon as it is final (spread across the
            # block so the DVE queue never sees a burst)
            if j in DVE_DEN:
                gs = groups(j)
                if g == gs[0]:
                    acc_of[j] = accp.tile([CK, 2 * QB], BF16, tag="acc",
                                          name="acc")
                acc = acc_of[j]
                if g == gs[0]:
                    if diag:
                        # diag-B covers queries 256:512 -> acc cols 768:1024
                        nc.vector.tensor_copy(acc[:, 0:QB], pt[:, 0:QB])
                        nc.vector.memset(acc[:, QB:QB + 256], 0.0)
                        nc.vector.tensor_copy(acc[:, QB + 256:2 * QB],
                                              pt[:, QB:QB + 256])
                    else:
                        nc.vector.tensor_copy(acc[:], pt[:])
                elif diag:
                    nc.vector.tensor_add(acc[:, 0:QB], acc[:, 0:QB],
                                         pt[:, 0:QB])
                    nc.vector.tensor_add(acc[:, QB + 256:2 * QB],
                                         acc[:, QB + 256:2 * QB],
                                         pt[:, QB:QB + 256])
                else:
                    nc.vector.tensor_add(acc[:], acc[:], pt[:])

        def emit_PV(u):
            j, g = units[u]
            qs = slice(j * QB, (j + 1) * QB)
            ka, kb = 2 * g, 2 * g + 1
            diag = (g == j)
            if j == JORDER[-1]:
                first = (g == j)
                last = (g == j - 1)
            else:
                first = (g == 0)
                last = (g == j)
            if first:
                po_of[j] = ps_o.tile([D, QB], FP32, tag="o", name="po")
            po = po_of[j]
            pt = pt_of[u]
            if diag:
                # full-range A first (start clears the tile), then the
                # column-restricted B accumulation
                nc.tensor.matmul(po[:], t_V[:, ka * D:(ka + 1) * D],
                                 pt[:, 0:QB], start=first, stop=False)
                nc.tensor.matmul(po[:, 256:QB], t_V[:, kb * D:(kb + 1) * D],
                                 pt[:, QB:QB + 256],
                                 start=False, stop=last)
            else:
                nc.tensor.matmul(po[:], t_V[:, ka * D:(ka + 1) * D],
                                 pt[:, 0:QB], start=first, stop=False)
                nc.tensor.matmul(po[:], t_V[:, kb * D:(kb + 1) * D],
                                 pt[:, QB:2 * QB],
                                 start=False, stop=last)
            if last:              # drain out^T as soon as its group closes
                so = stage.tile([D, QB], BF16, tag="so")
                nc.vector.tensor_copy(so[:], po[:])
                nc.sync.dma_start(outT[:, qs], so[:])

        def emit_DEN(j):
            qs = slice(j * QB, (j + 1) * QB)
            pd_ = ps_d.tile([1, QB], FP32, tag="d", name="pd")
            if j in DVE_DEN:
                # the DVE accumulator holds the block's exp sum; two small
                # matmuls reduce it across partitions
                acc = acc_of.pop(j)
                nc.tensor.matmul(pd_[:], t_ones, acc[:, 0:QB],
                                 start=True, stop=False)
                nc.tensor.matmul(pd_[:], t_ones, acc[:, QB:2 * QB],
                                 start=False, stop=True)
                for g in groups(j):
                    pt_of.pop(units.index((j, g)))
            else:
                # one burst of 1-col-stationary matmuls over the block's
                # kept pt tiles; single LDW stall, low array power
                gs = groups(j)
                for i, g in enumerate(gs):
                    u = units.index((j, g))
                    pt = pt_of.pop(u)
                    diag = (g == j)
                    first = (i == 0)
                    last = (i == len(gs) - 1)
                    if diag:
                        nc.tensor.matmul(pd_[:], t_ones, pt[:, 0:QB],
                                         start=first, stop=False)
                        nc.tensor.matmul(pd_[:, 256:QB], t_ones,
                                         pt[:, QB:QB + 256],
                                         start=False, stop=last)
                    else:
                        nc.tensor.matmul(pd_[:], t_ones, pt[:, 0:QB],
                                         start=first, stop=False)
                        nc.tensor.matmul(pd_[:], t_ones, pt[:, QB:2 * QB],
                                         start=False, stop=last)
            sd = stage.tile([1, QB], FP32, tag="sd")
            nc.vector.tensor_copy(sd[:], pd_[:])
            nc.scalar.dma_start(den[0:1, qs], sd[:])

        LOOKAHEAD = 3
        started = set()
        done_pv = 0

        def pv_done(u):
            nonlocal done_pv
            emit_PV(u)
            done_pv = u + 1
            j, g = units[u]
            lastu = (g == j - 1) if j == JORDER[-1] else (g == j)
            if lastu:
                emit_DEN(j)

        for u in range(len(units)):
            j, g = units[u]
            if j not in started:
                started.add(j)
                for p in projs_at.get(j, []):
                    p()
            emit_S(u)
            if u >= LOOKAHEAD:
                pv_done(u - LOOKAHEAD)
        for u in range(len(units) - LOOKAHEAD, len(units)):
            pv_done(u)

    nc.compile()
    return nc


def get_nc():
    if "nc" not in _CACHE:
        _CACHE["nc"] = _build()
    return _CACHE["nc"]


def make_in_maps(x, Wq, Wk, Wv):
    x = np.asarray(x, dtype=np.float32)
    wqT = np.asarray(Wq, dtype=np.float32).T
    wkT = np.asarray(Wk, dtype=np.float32).T
    wvT = np.asarray(Wv, dtype=np.float32).T
    wpack = np.ascontiguousarray(
        np.concatenate([wqT, wkT, wvT], axis=1).astype(np.float16))

    kk = np.arange(CK)[:, None]
    qq = np.arange(QB)[None, :]
    in_maps = []
    for c in range(NCORES):
        b, f = c // 2, c % 2
        xb = x[b]                       # [S, D]
        xqT = np.ascontiguousarray(xb.T.astype(np.float16))
        rows = (np.arange(S // 2) // CK) * 256 + CK * f + (np.arange(S // 2) % CK)
        xkvT = np.ascontiguousarray(xb[rows].T.astype(np.float16))
        maskA = (qq - kk >= CK * f).astype(bfloat16)
        in_maps.append({
            "xqT": xqT, "xkvT": xkvT,
            "wpack": wpack,
            "mpack": np.ascontiguousarray(maskA),
        })
    return in_maps


def combine(results):
    out = np.empty((B, S, D), np.float32)
    for b in range(B):
        o0 = results[2 * b]["outT"].astype(np.float64)
        o1 = results[2 * b + 1]["outT"].astype(np.float64)
        d0 = results[2 * b]["den"].astype(np.float64)
        d1 = results[2 * b + 1]["den"].astype(np.float64)
        out[b] = (((o0 + o1) / (d0 + d1)).T).astype(np.float32)
    return out


def kernel(x, Wq, Wk, Wv):
    nc = get_nc()
    in_maps = make_in_maps(x, Wq, Wk, Wv)
    res = run_bass_kernel_spmd(nc, in_maps, core_ids=list(range(NCORES)))
    return combine(res.results)


if __name__ == "__main__":
    import reference
    inputs = reference.setup_inputs()
    expected = np.asarray(reference.reference(**inputs))
    actual = kernel(**{k: np.asarray(v) for k, v in inputs.items()})
    err = np.abs(actual - expected).max()
    print("absmax err:", err, " scale:", np.abs(expected).max())


# revision 29
# speedup vs baseline: 1.0655x; 1.0655x over previous
"""Single-head causal attention (B=4, S=4096, D=128, fp32) on 8 Trainium2 cores.

Sharding: core c = (batch b = c//2, fold f = c%2). Each core processes ALL
queries of its batch but only the keys in 128-row chunks at global offsets
256*k + 128*f (k = 0..15). This interleaving makes the causal schedule
identical on every core (SPMD requires one program), while host-side input
prep (pre-transposed x, compacted kv rows, mask tiles passed as data) absorbs
all per-core differences into the input data.

Device outputs per core: unnormalized out^T partial [128, 4096] (bf16) and
softmax denominator partial [1, 4096] (fp32). Host combines the two folds per
batch: out[b] = ((outT0 + outT1) / (den0 + den1)).T

Dtypes: x/W/Q/K are fp16 (best mantissa for ~N(0,1) data, halves the DMA
head); exp weights / V / masks / out partials are bf16 (fp32 exponent range —
the unnormalized partials reach ~1e5 which overflows fp16). All matmuls run
at the same 1 col/cycle PE rate; 16-bit SBUF tiles unlock DVE 2x modes.

Diagonal units exploit causality: the second key chunk of a diagonal unit is
fully masked for queries 0..255 of the block on both folds, so S^T/exp/PV/den
all restrict it to columns 256:512. Its mask equals maskA[:, 0:256], so only
one [128, 512] mask tile is uploaded.

Denominator matmuls (1-column all-ones stationary) are batched per query
block: a partial-width LDWEIGHTS cannot pull ahead under an in-flight
full-width matmul (~90ns exposed stall), so paying that once per block
instead of once per unit saves ~2.5us. A full-width all-ones stationary
would avoid the stall but lights up the whole MAC array and trips the P0
power-state downclock (PE 2.4 -> 2.0 GHz; measured).
"""

import numpy as np
from contextlib import ExitStack
from ml_dtypes import bfloat16

import concourse.bacc as bacc
import concourse.tile as tile
import concourse.mybir as mybir
from concourse.bass_utils import run_bass_kernel_spmd

B, S, D = 4, 4096, 128
NCORES = 8
QB = 512          # query block (matmul moving dim)
CK = 128          # key chunk (matmul stationary dim)
NQB = S // QB     # 8 query blocks
NCK = 16          # key chunks per core (S/2/CK)
SCALE = float(1.0 / np.sqrt(D))
WARMUP_MMS = 3    # dummy matmuls to ramp the PE clock during the DMA head
DVE_DEN = {4, 5, 6, 7}  # blocks whose denominator accumulates on the DVE
                        # instead of streaming every exp tile through the PE
                        # a second time

# packed input layout (columns of the single [128, 7040] fp16 upload);
# ordered by first use so a handful of large DMAs deliver data just in time
C_WQ = 0
C_WK = 128
C_WV = 256
C_XKV0 = 384        # xkv cols    0:512
C_XQ1 = 896         # xq  cols  512:1024
C_MASK = 1408       # maskA (bf16 bits) [128, 512]
C_XKV1 = 1920       # xkv cols  512:1024
C_XQ2 = 2432        # xq  cols 1024:1536
C_XKV2 = 2944       # xkv cols 1024:2048
C_XQ3 = 3968        # xq  cols 1536:2048
C_XQ0 = 4480        # xq  cols    0:512
C_XQ47 = 4992       # xq  cols 2048:4096
C_END = 7040
XQ_BASE = {0: C_XQ0, 1: C_XQ1, 2: C_XQ2, 3: C_XQ3,
           4: C_XQ47, 5: C_XQ47 + 512, 6: C_XQ47 + 1024, 7: C_XQ47 + 1536}


def xkv_base(c):
    """Map xkv column c (0..2047) to its packed-input column."""
    if c < 512:
        return C_XKV0 + c
    if c < 1024:
        return C_XKV1 + (c - 512)
    return C_XKV2 + (c - 1024)

FP32 = mybir.dt.float32
FP16 = mybir.dt.float16
BF16 = mybir.dt.bfloat16

_CACHE = {}


def _build():
    nc = bacc.Bacc("TRN2", target_bir_lowering=False, debug=False)

    inp = nc.dram_tensor("inp", [D, C_END], FP16, kind="ExternalInput").ap()

    outT = nc.dram_tensor("outT", [D, S], BF16, kind="ExternalOutput").ap()
    den = nc.dram_tensor("den", [1, S], FP32, kind="ExternalOutput").ap()

    with tile.TileContext(nc) as tc, ExitStack() as ctx:
        consts = ctx.enter_context(tc.tile_pool(name="consts", bufs=1))
        stage = ctx.enter_context(tc.tile_pool(name="stage", bufs=2))
        ptp = ctx.enter_context(tc.tile_pool(name="ptp", bufs=12))
        accp = ctx.enter_context(tc.tile_pool(name="accp", bufs=2))
        ps_s = ctx.enter_context(tc.tile_pool(name="ps_s", bufs=3, space="PSUM"))
        ps_o = ctx.enter_context(tc.tile_pool(name="ps_o", bufs=1, space="PSUM"))
        ps_d = ctx.enter_context(tc.tile_pool(name="ps_d", bufs=1, space="PSUM"))

        # ---- PE warm-up: dummy matmuls on zeroed scratch, no load deps ----
        t_z = consts.tile([D, QB], FP16, tag="z")
        nc.vector.memset(t_z[:], 0.0)
        t_ones = consts.tile([CK, 1], BF16, tag="ones")
        nc.gpsimd.memset(t_ones[:], 1.0)
        # dummy activation: hoists the one-time ~2.7us exp table load off
        # the first real exp (which sits on the critical PE->ACT->PE path)
        t_warm = consts.tile([CK, 8], BF16, tag="actwarm")
        nc.scalar.activation(t_warm[:], t_z[:, 0:8],
                             mybir.ActivationFunctionType.Exp)

        def dummy_mm():
            pz = ps_s.tile([CK, 2 * QB], FP32, tag="s", name="pz")
            nc.tensor.matmul(pz[:, 0:QB], t_z[:, 0:CK], t_z[:],
                             start=True, stop=True)

        for _ in range(WARMUP_MMS):
            dummy_mm()

        # ---- loads: a few large DMAs over two parallel queues. Per-DMA
        # descriptor generation + doorbell latency (~1.3-2us each) dominates
        # the startup, so fewer/bigger transfers land data much earlier. ----
        t_inp = consts.tile([D, C_END], FP16, tag="inp")

        nc.sync.dma_start(t_inp[:, 0:C_XQ1], inp[:, 0:C_XQ1])
        nc.gpsimd.dma_start(t_inp[:, C_XQ1:C_XKV1], inp[:, C_XQ1:C_XKV1])
        nc.sync.dma_start(t_inp[:, C_XKV1:C_XKV2], inp[:, C_XKV1:C_XKV2])
        nc.gpsimd.dma_start(t_inp[:, C_XKV2:C_XQ0], inp[:, C_XKV2:C_XQ0])
        nc.sync.dma_start(t_inp[:, C_XQ0:C_XQ47], inp[:, C_XQ0:C_XQ47])
        nc.gpsimd.dma_start(t_inp[:, C_XQ47:C_END], inp[:, C_XQ47:C_END])

        t_wq = t_inp[:, C_WQ:C_WQ + D]
        t_wk = t_inp[:, C_WK:C_WK + D]
        t_wv = t_inp[:, C_WV:C_WV + D]
        t_mA = t_inp[:, C_MASK:C_MASK + QB].bitcast(BF16)

        # ---- projections (emitted interleaved with attention blocks, in the
        # order the DMA queue delivers their inputs) ----
        t_KT = consts.tile([D, S // 2], FP16, tag="KT")
        t_QT = consts.tile([D, S], FP16, tag="QT")
        t_V = consts.tile([CK, NCK * D], BF16, tag="V")

        def proj_kt(t, halves=False):   # 512 compacted keys = chunks 4t..4t+3
            pk = ps_s.tile([D, 2 * QB], FP32, tag="s")
            base = xkv_base(t * QB)
            if halves:
                for h in range(2):
                    sl = slice(base + h * 256, base + (h + 1) * 256)
                    nc.tensor.matmul(pk[:, h * 256:(h + 1) * 256], t_wk,
                                     t_inp[:, sl], start=True, stop=True)
            else:
                nc.tensor.matmul(pk[:, 0:QB], t_wk,
                                 t_inp[:, base:base + QB],
                                 start=True, stop=True)
            nc.vector.tensor_copy(t_KT[:, t * QB:(t + 1) * QB], pk[:, 0:QB])

        def proj_qt(t):   # query block t
            pq = ps_s.tile([D, 2 * QB], FP32, tag="s")
            base = XQ_BASE[t]
            nc.tensor.matmul(pq[:, 0:QB], t_wq, t_inp[:, base:base + QB],
                             start=True, stop=True)
            nc.vector.tensor_copy(t_QT[:, t * QB:(t + 1) * QB], pq[:, 0:QB])

        def proj_v(t):    # chunks 4t..4t+3
            pv = ps_s.tile([D, 2 * QB], FP32, tag="s")
            for h in range(4):
                k = 4 * t + h
                base = xkv_base(k * CK)
                nc.tensor.matmul(pv[:, h * D:(h + 1) * D],
                                 t_inp[:, base:base + CK], t_wv,
                                 start=True, stop=True)
            nc.vector.tensor_copy(t_V[:, t * QB:(t + 1) * QB], pv[:, 0:QB])

        # ---- attention: flat unit stream, software-pipelined so each
        # unit's PV matmuls are emitted after the NEXT unit's S^T matmuls
        # (PE is in-order; this hides the exp latency). Denominator matmuls
        # are batched per block (see module docstring). ----
        JORDER = [1, 2, 3, 0, 4, 5, 6, 7]
        # last block runs its diagonal group first so the final unit has no
        # mask-mul chain in the tail
        def groups(j):
            if j == JORDER[-1]:
                return [j] + list(range(j))
            return list(range(j + 1))
        units = [(j, g) for j in JORDER for g in groups(j)]
        pt_of = {}
        po_of = {}
        acc_of = {}

        # projections interleaved at block starts, matching DMA arrival
        projs_at = {
            1: [lambda: proj_kt(0, halves=True), lambda: proj_qt(1),
                lambda: proj_v(0)],
            2: [lambda: proj_kt(1), lambda: proj_qt(2), lambda: proj_v(1)],
            3: [lambda: proj_kt(2), lambda: proj_kt(3), lambda: proj_qt(3),
                lambda: proj_v(2), lambda: proj_v(3)],
            0: [lambda: proj_qt(0)],
            4: [lambda: proj_qt(4)],
            5: [lambda: proj_qt(5)],
            6: [lambda: proj_qt(6)],
            7: [lambda: proj_qt(7)],
        }

        # Diagonal unit geometry: chunk B (key offset 256+128f within the
        # block) is fully masked for block-local queries 0..255 on both
        # folds, so its scores/exp/PV/den cover only query columns 256:512.
        # Its surviving mask equals maskA[:, 0:256].
        def emit_S(u):
            j, g = units[u]
            qs = slice(j * QB, (j + 1) * QB)
            ka, kb = 2 * g, 2 * g + 1
            diag = (g == j)
            pst = ps_s.tile([CK, 2 * QB], FP32, tag="s")
            nc.tensor.matmul(pst[:, 0:QB],
                             t_KT[:, ka * CK:(ka + 1) * CK], t_QT[:, qs],
                             start=True, stop=True)
            if diag:
                qsB = slice(j * QB + 256, (j + 1) * QB)
                nc.tensor.matmul(pst[:, QB:QB + 256],
                                 t_KT[:, kb * CK:(kb + 1) * CK], t_QT[:, qsB],
                                 start=True, stop=True)
                wid = QB + 256
            else:
                nc.tensor.matmul(pst[:, QB:2 * QB],
                                 t_KT[:, kb * CK:(kb + 1) * CK], t_QT[:, qs],
                                 start=True, stop=True)
                wid = 2 * QB
            pt = ptp.tile([CK, 2 * QB], BF16, tag="pt")
            nc.scalar.activation(pt[:, 0:wid], pst[:, 0:wid],
                                 mybir.ActivationFunctionType.Exp,
                                 scale=SCALE)
            if diag:
                nc.vector.tensor_mul(pt[:, 0:QB], pt[:, 0:QB], t_mA)
                nc.vector.tensor_mul(pt[:, QB:QB + 256], pt[:, QB:QB + 256],
                                     t_mA[:, 0:256])
            pt_of[u] = pt
            # DVE-denominator blocks: fold this unit's exp tile into the
            # block accumulator as soon as it is final (spread across the
            # block so the DVE queue never sees a burst)
            if j in DVE_DEN:
                gs = groups(j)
                if g == gs[0]:
                    acc_of[j] = accp.tile([CK, 2 * QB], BF16, tag="acc",
                                          name="acc")
                acc = acc_of[j]
                if g == gs[0]:
                    if diag:
                        # diag-B covers queries 256:512 -> acc cols 768:1024
                        nc.vector.tensor_copy(acc[:, 0:QB], pt[:, 0:QB])
                        nc.vector.memset(acc[:, QB:QB + 256], 0.0)
                        nc.vector.tensor_copy(acc[:, QB + 256:2 * QB],
                                              pt[:, QB:QB + 256])
                    else:
                        nc.vector.tensor_copy(acc[:], pt[:])
                elif diag:
                    nc.vector.tensor_add(acc[:, 0:QB], acc[:, 0:QB],
                                         pt[:, 0:QB])
                    nc.vector.tensor_add(acc[:, QB + 256:2 * QB],
                                         acc[:, QB + 256:2 * QB],
                                         pt[:, QB:QB + 256])
                else:
                    nc.vector.tensor_add(acc[:], acc[:], pt[:])

        def emit_PV(u):
            j, g = units[u]
            qs = slice(j * QB, (j + 1) * QB)
            ka, kb = 2 * g, 2 * g + 1
            diag = (g == j)
            if j == JORDER[-1]:
                first = (g == j)
                last = (g == j - 1)
            else:
                first = (g == 0)
                last = (g == j)
            if first:
                po_of[j] = ps_o.tile([D, QB], FP32, tag="o", name="po")
            po = po_of[j]
            pt = pt_of[u]
            if diag:
                # full-range A first (start clears the tile), then the
                # column-restricted B accumulation
                nc.tensor.matmul(po[:], t_V[:, ka * D:(ka + 1) * D],
                                 pt[:, 0:QB], start=first, stop=False)
                nc.tensor.matmul(po[:, 256:QB], t_V[:, kb * D:(kb + 1) * D],
                                 pt[:, QB:QB + 256],
                                 start=False, stop=last)
            else:
                nc.tensor.matmul(po[:], t_V[:, ka * D:(ka + 1) * D],
                                 pt[:, 0:QB], start=first, stop=False)
                nc.tensor.matmul(po[:], t_V[:, kb * D:(kb + 1) * D],
                                 pt[:, QB:2 * QB],
                                 start=False, stop=last)
            if last:              # drain out^T as soon as its group closes
                so = stage.tile([D, QB], BF16, tag="so")
                nc.vector.tensor_copy(so[:], po[:])
                nc.sync.dma_start(outT[:, qs], so[:])

        def emit_DEN(j):
            qs = slice(j * QB, (j + 1) * QB)
            pd_ = ps_d.tile([1, QB], FP32, tag="d", name="pd")
            if j in DVE_DEN:
                # the DVE accumulator holds the block's exp sum; two small
                # matmuls reduce it across partitions
                acc = acc_of.pop(j)
                nc.tensor.matmul(pd_[:], t_ones, acc[:, 0:QB],
                                 start=True, stop=False)
                nc.tensor.matmul(pd_[:], t_ones, acc[:, QB:2 * QB],
                                 start=False, stop=True)
                for g in groups(j):
                    pt_of.pop(units.index((j, g)))
            else:
                # one burst of 1-col-stationary matmuls over the block's
                # kept pt tiles; single LDW stall, low array power
                gs = groups(j)
                for i, g in enumerate(gs):
                    u = units.index((j, g))
                    pt = pt_of.pop(u)
                    diag = (g == j)
                    first = (i == 0)
                    last = (i == len(gs) - 1)
                    if diag:
                        nc.tensor.matmul(pd_[:], t_ones, pt[:, 0:QB],
                                         start=first, stop=False)
                        nc.tensor.matmul(pd_[:, 256:QB], t_ones,
                                         pt[:, QB:QB + 256],
                                         start=False, stop=last)
                    else:
                        nc.tensor.matmul(pd_[:], t_ones, pt[:, 0:QB],
                                         start=first, stop=False)
                        nc.tensor.matmul(pd_[:], t_ones, pt[:, QB:2 * QB],
                                         start=False, stop=last)
            sd = stage.tile([1, QB], FP32, tag="sd")
            nc.vector.tensor_copy(sd[:], pd_[:])
            nc.scalar.dma_start(den[0:1, qs], sd[:])

        LOOKAHEAD = 3
        started = set()
        done_pv = 0

        def pv_done(u):
            nonlocal done_pv
            emit_PV(u)
            done_pv = u + 1
            j, g = units[u]
            lastu = (g == j - 1) if j == JORDER[-1] else (g == j)
            if lastu:
                emit_DEN(j)

        for u in range(len(units)):
            j, g = units[u]
            if j not in started:
                started.add(j)
                for p in projs_at.get(j, []):
                    p()
            emit_S(u)
            if u >= LOOKAHEAD:
                pv_done(u - LOOKAHEAD)
        for u in range(len(units) - LOOKAHEAD, len(units)):
            pv_done(u)

    nc.compile()
    return nc


def get_nc():
    if "nc" not in _CACHE:
        _CACHE["nc"] = _build()
    return _CACHE["nc"]


def make_in_maps(x, Wq, Wk, Wv):
    x = np.asarray(x, dtype=np.float32)
    wqT = np.asarray(Wq, dtype=np.float32).T.astype(np.float16)
    wkT = np.asarray(Wk, dtype=np.float32).T.astype(np.float16)
    wvT = np.asarray(Wv, dtype=np.float32).T.astype(np.float16)

    kk = np.arange(CK)[:, None]
    qq = np.arange(QB)[None, :]
    in_maps = []
    for c in range(NCORES):
        b, f = c // 2, c % 2
        xb = x[b]                       # [S, D]
        xqT = xb.T.astype(np.float16)
        rows = (np.arange(S // 2) // CK) * 256 + CK * f + (np.arange(S // 2) % CK)
        xkvT = xb[rows].T.astype(np.float16)
        mask_bits = (qq - kk >= CK * f).astype(bfloat16).view(np.float16)
        packed = np.concatenate([
            wqT, wkT, wvT,
            xkvT[:, 0:512], xqT[:, 512:1024], mask_bits,
            xkvT[:, 512:1024], xqT[:, 1024:1536],
            xkvT[:, 1024:2048], xqT[:, 1536:2048],
            xqT[:, 0:512], xqT[:, 2048:4096],
        ], axis=1)
        assert packed.shape == (D, C_END)
        in_maps.append({"inp": np.ascontiguousarray(packed)})
    return in_maps


def combine(results):
    out = np.empty((B, S, D), np.float32)
    for b in range(B):
        o0 = results[2 * b]["outT"].astype(np.float64)
        o1 = results[2 * b + 1]["outT"].astype(np.float64)
        d0 = results[2 * b]["den"].astype(np.float64)
        d1 = results[2 * b + 1]["den"].astype(np.float64)
        out[b] = (((o0 + o1) / (d0 + d1)).T).astype(np.float32)
    return out


def kernel(x, Wq, Wk, Wv):
    nc = get_nc()
    in_maps = make_in_maps(x, Wq, Wk, Wv)
    res = run_bass_kernel_spmd(nc, in_maps, core_ids=list(range(NCORES)))
    return combine(res.results)


if __name__ == "__main__":
    import reference
    inputs = reference.setup_inputs()
    expected = np.asarray(reference.reference(**inputs))
    actual = kernel(**{k: np.asarray(v) for k, v in inputs.items()})
    err = np.abs(actual - expected).max()
    print("absmax err:", err, " scale:", np.abs(expected).max())
